# revision 1
# baseline (speedup 1.0000x reference)
"""int4 weight-only quantized GEMV on 8 TRN2 NeuronCores - TensorEngine version.

out[1, n] = sum_k A[1, k] * W[n, k],   W[n,k] = (nib[n,k] - 8) * s[n,g] + z[n,g]
A: [1, 8192] fp16, B: [16384, 4096] int32 (one byte per elem, 2 nibbles),
scalesAndZeros: [16384, 256, 2] fp16 (group=32 along K).

Sharding: N=16384 rows split across 8 cores (ns=2048 each); A replicated
(baked into per-core stationaries).

Math:  out[n] = sum_g s[n,g]*dotg[n,g] + W2[n]
       dotg[n,g] = sum_{j in g} lo_j*Ae_j + hi_j*Ao_j   (j = byte col, 16/group)
       W2[n] = sum_g sA_g*(z[n,g] - 8*s[n,g])           (host, exact)

Device: host pre-expands nibbles to fp8e4 streams LO/HI, K-major, chunked
[pair, p, nchunk, i, 512] so each matmul rhs is contiguous; PE fp8
DoubleRow matmuls with block-diagonal A stationaries (A = A1 + A2/16,
po rows 0-63 = A1, 64-127 = A2) accumulate group dots into PSUM; DVE
multiplies by per-virtual-group scales S2 (A2 rows derived on device);
a ones-stationary matmul reduces over groups; W2 added at the end.
"""

import numpy as np
import ml_dtypes

import concourse.bass as bass
import concourse.bacc as bacc
import concourse.mybir as mybir
from concourse import tile
from concourse.bass_utils import run_bass_kernel_spmd

FP16 = mybir.dt.float16
FP32 = mybir.dt.float32
FP8 = mybir.dt.float8e4
Alu = mybir.AluOpType
PM = mybir.MatmulPerfMode
F8NP = ml_dtypes.float8_e4m3

M, K, N = 1, 8192, 16384
KH = K // 2            # 4096 byte-columns
GROUP = 32             # k per group -> 16 bytes per group
NG = K // GROUP        # 256 groups
NCORES = 8
NS = N // NCORES       # 2048 rows per core
P = 128
NPAIR = KH // 256      # 16 slab-pairs (256 byte-rows each)
NSB = 4                # pair-superblocks (4 pairs -> 128 virtual po rows)
NQ = NS // 512         # 4 n-chunks of 512


def build_program(ns=NS):
    nc = bacc.Bacc()
    lo_d = nc.declare_dram_parameter("LO8", [NPAIR, P, NQ, 2, 512], FP8, isOutput=False)
    hi_d = nc.declare_dram_parameter("HI8", [NPAIR, P, NQ, 2, 512], FP8, isOutput=False)
    s2_d = nc.declare_dram_parameter("S2", [NSB, 64, ns], FP16, isOutput=False)
    sta_d = nc.declare_dram_parameter("STA", [P, NPAIR, 2, 2, 128], FP8, isOutput=False)
    w2_d = nc.declare_dram_parameter("W2", [1, ns], FP32, isOutput=False)
    out_d = nc.declare_dram_parameter("OUT", [1, ns], FP16, isOutput=True)

    with tile.TileContext(nc) as tc:
        with (
            tc.tile_pool(name="const", bufs=1) as cpool,
            tc.tile_pool(name="stream", bufs=1) as strpool,
            tc.tile_pool(name="work", bufs=4) as wpool,
            tc.tile_pool(name="ps", bufs=3, space="PSUM") as pspool,
            tc.tile_pool(name="pso", bufs=1, space="PSUM") as psopool,
        ):
            sta = cpool.tile([P, NPAIR, 2, 2, 128], FP8)
            nc.sync.dma_start(out=sta[:], in_=sta_d[:])
            ones = cpool.tile([P, 1], FP16)
            nc.gpsimd.memset(ones[:], 1.0)
            w2 = cpool.tile([1, ns], FP32)
            nc.sync.dma_start(out=w2[:, :], in_=w2_d[:, :])
            s2 = []
            for sb in range(NSB):
                t_ = cpool.tile([P, ns], FP16, tag=f"s2_{sb}")
                nc.scalar.dma_start(out=t_[0:64, :], in_=s2_d[sb])
                nc.vector.tensor_scalar(out=t_[64:128, :], in0=t_[0:64, :],
                                        scalar1=0.0625, scalar2=None, op0=Alu.mult)
                s2.append(t_)

            # streams: pairs 0-11 as 512KB DMAs; pairs 12-15 (last
            # superblock) chunked per n-quarter so the tail is shallow
            lot = [[None] * NQ for _ in range(NPAIR)]
            hit = [[None] * NQ for _ in range(NPAIR)]
            for t in range(15):
                lt = strpool.tile([P, NQ, 2, 512], FP8, tag=f"lo{t}")
                nc.sync.dma_start(out=lt[:], in_=lo_d[t])
                ht = strpool.tile([P, NQ, 2, 512], FP8, tag=f"hi{t}")
                nc.scalar.dma_start(out=ht[:], in_=hi_d[t])
                for q in range(NQ):
                    lot[t][q] = lt[:, q]
                    hit[t][q] = ht[:, q]
            for q in range(NQ):
                for t in range(15, NPAIR):
                    lt = strpool.tile([P, 2, 512], FP8, tag=f"lo{t}q{q}")
                    nc.sync.dma_start(out=lt[:], in_=lo_d[t, :, q])
                    ht = strpool.tile([P, 2, 512], FP8, tag=f"hi{t}q{q}")
                    nc.scalar.dma_start(out=ht[:], in_=hi_d[t, :, q])
                    lot[t][q] = lt[:]
                    hit[t][q] = ht[:]

            psout = psopool.tile([P, ns], FP32, tag="psout")
            pending = []

            def emit_ones(sb, q, e):
                nc.tensor.matmul(
                    out=psout[0:1, 512 * q : 512 * q + 512],
                    lhsT=ones[:, :], rhs=e[:, :],
                    start=(sb == 0), stop=(sb == NSB - 1),
                )
                if sb == NSB - 1:
                    outt = wpool.tile([1, 512], FP16, tag=f"outt{q}")
                    nc.vector.tensor_tensor(
                        out=outt[:, :],
                        in0=psout[0:1, 512 * q : 512 * q + 512],
                        in1=w2[:, 512 * q : 512 * q + 512], op=Alu.add,
                    )
                    nc.gpsimd.dma_start(
                        out=out_d[0:1, 512 * q : 512 * q + 512],
                        in_=outt[0:1, :])

            for sb in range(NSB):
                for q in range(NQ):
                    ps = pspool.tile([P, 512], FP32, tag="ps")
                    for c in range(4):
                        t = 4 * sb + c
                        nc.tensor.matmul(
                            out=ps[:, :], lhsT=sta[:, t, 0], rhs=lot[t][q],
                            start=(c == 0), stop=False,
                            perf_mode=PM.DoubleRow,
                        )
                        nc.tensor.matmul(
                            out=ps[:, :], lhsT=sta[:, t, 1], rhs=hit[t][q],
                            start=False, stop=(c == 3),
                            perf_mode=PM.DoubleRow,
                        )
                    e = wpool.tile([P, 512], FP16, tag="e")
                    nc.vector.tensor_tensor(
                        out=e[:, :], in0=ps[:, :],
                        in1=s2[sb][:, 512 * q : 512 * q + 512], op=Alu.mult,
                    )
                    pending.append((sb, q, e))
                    # flush ones-matmuls two units behind: their TT is done,
                    # so the PE never blocks on the Vector engine
                    if len(pending) > 2:
                        emit_ones(*pending.pop(0))
            for item in pending:
                emit_ones(*item)
    nc.finalize()
    return nc


_NC_CACHE = {}


def _get_program(ns=NS):
    if ns not in _NC_CACHE:
        _NC_CACHE[ns] = build_program(ns)
    return _NC_CACHE[ns]


def _split_fp8(c):
    """c (fp32 array) -> (A1, A2) fp8 with c ~ A1 + A2/16."""
    a1 = c.astype(F8NP)
    resid = (c - a1.astype(np.float32)) * 16.0
    a2 = resid.astype(F8NP)
    return a1, a2


def prep_inputs(A, B, scalesAndZeros):
    """Host prep: nibble->fp8 streams, stationaries, scales, W2."""
    A = np.asarray(A).reshape(K).astype(np.float32)
    B = np.asarray(B)
    SZ = np.asarray(scalesAndZeros)

    # fp8 nibble LUT expansion, transposed to [KH, N]
    b8 = B.astype(np.uint8)              # [N, KH]
    lut = np.arange(16, dtype=np.float32).astype(F8NP)  # exact
    lo8 = lut[b8 & 15]                   # [N, KH] fp8
    hi8 = lut[b8 >> 4]
    lo8_t = np.ascontiguousarray(lo8.T)  # [KH, N]
    hi8_t = np.ascontiguousarray(hi8.T)

    # stationaries: per byte-row kb: lo coef Ae=A[2kb], hi coef Ao=A[2kb+1]
    ae = A[0::2]
    ao = A[1::2]
    ae1, ae2 = _split_fp8(ae)
    ao1, ao2 = _split_fp8(ao)
    sta = np.zeros((P, NPAIR, 2, 2, 128), F8NP)
    kb = np.arange(KH)
    tt, ii, pp, uu = kb // 256, (kb // 128) % 2, kb % 128, (kb // 16) % 16
    band = tt % 4  # po: A1 rows 0..63 (16*band+u), A2 rows 64..127
    sta[pp, tt, 0, ii, 16 * band + uu] = ae1[kb]
    sta[pp, tt, 0, ii, 64 + 16 * band + uu] = ae2[kb]
    sta[pp, tt, 1, ii, 16 * band + uu] = ao1[kb]
    sta[pp, tt, 1, ii, 64 + 16 * band + uu] = ao2[kb]

    s = SZ[..., 0].astype(np.float32)    # [N, NG]
    z = SZ[..., 1].astype(np.float32)
    sag = A.reshape(NG, GROUP).sum(-1, dtype=np.float64).astype(np.float32)
    w2_full = (sag[None, :] * (z - 8.0 * s)).sum(-1, dtype=np.float64).astype(np.float32)

    s2_full = np.zeros((NSB, 64, N), np.float16)
    for sb in range(NSB):
        for c in range(4):
            t = 4 * sb + c
            g0 = 16 * t
            s_blk = s[:, g0 : g0 + 16].T          # [16, N]
            s2_full[sb, 16 * c : 16 * c + 16] = s_blk.astype(np.float16)

    in_maps = []
    for core in range(NCORES):
        n0, n1 = core * NS, (core + 1) * NS
        # [kb, n] -> [t, p, q, i, f]
        # [kb, n] -> [t, p, q, i, f]
        lo_arr = lo8_t[:, n0:n1].reshape(NPAIR, 2, P, NQ, 512).transpose(0, 2, 3, 1, 4)
        hi_arr = hi8_t[:, n0:n1].reshape(NPAIR, 2, P, NQ, 512).transpose(0, 2, 3, 1, 4)
        in_maps.append({
            "LO8": np.ascontiguousarray(lo_arr),
            "HI8": np.ascontiguousarray(hi_arr),
            "S2": np.ascontiguousarray(s2_full[:, :, n0:n1]),
            "STA": sta,
            "W2": w2_full[n0:n1].reshape(1, NS),
        })
    return in_maps


def kernel(A, B, scalesAndZeros):
    in_maps = prep_inputs(A, B, scalesAndZeros)
    nc = _get_program()
    res = run_bass_kernel_spmd(nc, in_maps, core_ids=list(range(NCORES)))
    out = np.concatenate([res.results[c]["OUT"].reshape(NS) for c in range(NCORES)])
    return out.reshape(1, N).astype(np.float16)


if __name__ == "__main__":
    rng = np.random.default_rng(0)
    A = rng.standard_normal((M, K)).astype(np.float16)
    B = rng.integers(0, 256, (N, KH)).astype(np.int32)
    SZ = rng.standard_normal((N, NG, 2)).astype(np.float16)
    out = kernel(A, B, SZ)
    bb = B.astype(np.int64)
    q = np.stack([bb & 15, (bb >> 4) & 15], axis=-1).reshape(N, K).astype(np.float64) - 8.0
    s = SZ[..., 0].astype(np.float64)
    z = SZ[..., 1].astype(np.float64)
    W = (q.reshape(N, NG, GROUP) * s[:, :, None] + z[:, :, None]).reshape(N, K)
    exp = (A.astype(np.float64) @ W.T).astype(np.float16)
    err = np.abs(out.astype(np.float64) - exp.astype(np.float64))
    rel = err / np.maximum(np.abs(exp.astype(np.float64)), 1e-6)
    print("median rel:", np.median(rel), "absmax/scale:",
          err.max() / np.abs(exp).max())

